# revision 2
# baseline (speedup 1.0000x reference)
"""Trainium2 Bass kernel for the 4-layer soft-logic-gate cellular automaton.

Algorithm notes (hardcoded for x:(32,128,128), toggle_gates:(4,16,256,256)):

Sharding: spatial over state rows H=256, 8 cores, 32 output-state rows each,
with a redundant 4-row halo so there is NO cross-core communication.
Core i consumes x rows [16i, 16i+18) (mod 128) and gate rows [32i, 32i+35)
(mod 256), produces out rows [16i, 16i+16).

Per-pixel op: out = sum_c sigmoid(g_c) * prod_n (bit_n(c) ? v_n : 1-v_n) over
the 2x2 neighborhood. On device this is evaluated in the multilinear
(Moebius) basis: out = sum_{q,r} A_qr * n_q(row h) * n_r(row h+1) with
n = [1, v_right, v_left, v_left*v_right]; A = per-pixel Moebius transform of
the sigmoided gates (batch-free, computed once on device).

Layout: state split by column parity: SE/SO tiles [128 partitions = w',
free=(h local rows, b)]. w+1 for even cols = SO same partition; for odd cols
= SE at partition+1 (materialized once per layer via SBUF->SBUF DMA).
h+1 = free-dim offset. Layer 0 collapses (zero-upsampled input: 3 of 4
neighbors are zero). Layer 3 computes only even rows of even cols.
"""

import sys

sys.path.insert(0, "/opt/trn_rl_repo")

import numpy as np

import concourse.bacc as bacc
import concourse.mybir as mybir
from concourse.bass_utils import run_bass_kernel_spmd
from concourse.tile import TileContext

P = 128          # partitions = w' (column pairs)
B = 32           # batch
HS = 35          # local state rows (31 needed + 4 halo)
HT = 36          # tile rows (1 scratch row for h+1 reads)
HX = 18          # local x rows
NCORES = 8
F32 = mybir.dt.float32

_NC_CACHE = {}


def _build():
    nc = bacc.Bacc("TRN2", target_bir_lowering=False, debug=False, num_devices=NCORES)
    x_in = nc.dram_tensor("x", [P, HX, B], F32, kind="ExternalInput")
    g_in = nc.dram_tensor("g", [P, 2, 4, 4, 4, HS], F32, kind="ExternalInput")
    o_out = nc.dram_tensor("o", [P, 16, B], F32, kind="ExternalOutput")

    with TileContext(nc) as tc:
        with tc.tile_pool(name="pool", bufs=1) as pool:
            # ---- gate preprocessing: sigmoid + in-place Moebius transform ----
            gt = pool.tile([P, 2, 4, 4, 4, HS], F32, tag="gt")
            nc.sync.dma_start(out=gt[:], in_=g_in[:])
            sg = pool.tile([P, 2, 4, 4, 4, HS], F32, tag="sg")
            nc.scalar.activation(
                sg[:], gt[:], mybir.ActivationFunctionType.Sigmoid
            )
            # 4 bit-stages over (q,r) = (a0a1, a2a3): a[bit set] -= a[bit clear]
            s = sg
            nc.vector.tensor_sub(  # a3: odd r -= even r
                s[:, :, :, :, 1::2, :], s[:, :, :, :, 1::2, :], s[:, :, :, :, 0::2, :]
            )
            nc.vector.tensor_sub(  # a2: r in {2,3} -= r in {0,1}
                s[:, :, :, :, 2:4, :], s[:, :, :, :, 2:4, :], s[:, :, :, :, 0:2, :]
            )
            nc.vector.tensor_sub(  # a1: odd q -= even q
                s[:, :, :, 1::2, :, :], s[:, :, :, 1::2, :, :], s[:, :, :, 0::2, :, :]
            )
            nc.vector.tensor_sub(  # a0: q in {2,3} -= q in {0,1}
                s[:, :, :, 2:4, :, :], s[:, :, :, 2:4, :, :], s[:, :, :, 0:2, :, :]
            )

            def cof(ew, l, q, r, rows=slice(0, HS)):
                s = sg[:, ew, l, q, r, rows]
                n = s.shape[1]
                return s.unsqueeze(2).broadcast_to([P, n, B])

            # ---- x load + shifted copy ----
            xt = pool.tile([P, HX, B], F32, tag="xt")
            nc.sync.dma_start(out=xt[:], in_=x_in[:])
            xs = pool.tile([P, HX, B], F32, tag="xs")
            nc.sync.dma_start(out=xs[0:127], in_=xt[1:128])
            nc.sync.dma_start(out=xs[127:128], in_=xt[0:1])

            # ---- layer 0 (state_1 from zero-upsampled x) ----
            SE = pool.tile([P, HT, B], F32, tag="se1")
            SO = pool.tile([P, HT, B], F32, tag="so1")
            nc.scalar.memzero(SE[:, 34:HT, :])
            nc.scalar.memzero(SO[:, 34:HT, :])
            ev_c, od_c = slice(0, 33, 2), slice(1, 34, 2)  # coeff rows (17 each)
            t0 = pool.tile([P, 17, B], F32, tag="l0t")
            nc.vector.tensor_mul(t0[:], xt[:, 0:17, :], cof(0, 0, 2, 0, ev_c))
            nc.vector.tensor_add(SE[:, 0:34:2, :], t0[:], cof(0, 0, 0, 0, ev_c))
            t0 = pool.tile([P, 17, B], F32, tag="l0t")
            nc.vector.tensor_mul(t0[:], xt[:, 1:18, :], cof(0, 0, 0, 2, od_c))
            nc.vector.tensor_add(SE[:, 1:35:2, :], t0[:], cof(0, 0, 0, 0, od_c))
            t0 = pool.tile([P, 17, B], F32, tag="l0t")
            nc.vector.tensor_mul(t0[:], xs[:, 0:17, :], cof(1, 0, 1, 0, ev_c))
            nc.vector.tensor_add(SO[:, 0:34:2, :], t0[:], cof(1, 0, 0, 0, ev_c))
            t0 = pool.tile([P, 17, B], F32, tag="l0t")
            nc.vector.tensor_mul(t0[:], xs[:, 1:18, :], cof(1, 0, 0, 1, od_c))
            nc.vector.tensor_add(SO[:, 1:35:2, :], t0[:], cof(1, 0, 0, 0, od_c))

            # ---- layers 1, 2 (full) ----
            for l in (1, 2):
                SESH = pool.tile([P, HT, B], F32, tag="sesh", bufs=2)
                nc.sync.dma_start(out=SESH[0:127], in_=SE[1:128])
                nc.sync.dma_start(out=SESH[127:128], in_=SE[0:1])
                UE = pool.tile([P, HT, B], F32, tag="ue", bufs=2)
                UO = pool.tile([P, HT, B], F32, tag="uo", bufs=2)
                nc.scalar.memzero(UE[:, HS:HT, :])
                nc.scalar.memzero(UO[:, HS:HT, :])
                nc.vector.tensor_mul(UE[:, 0:HS, :], SE[:, 0:HS, :], SO[:, 0:HS, :])
                nc.vector.tensor_mul(UO[:, 0:HS, :], SO[:, 0:HS, :], SESH[:, 0:HS, :])
                SEn = pool.tile([P, HT, B], F32, tag=f"se{l + 1}")
                SOn = pool.tile([P, HT, B], F32, tag=f"so{l + 1}")
                nc.scalar.memzero(SEn[:, HS:HT, :])
                nc.scalar.memzero(SOn[:, HS:HT, :])
                for ew, (nR, nL, u, outt) in enumerate(
                    [(SO, SE, UE, SEn), (SESH, SO, UO, SOn)]
                ):
                    S = []
                    for r in range(4):
                        t1 = pool.tile([P, HS, B], F32, tag="t1", bufs=2)
                        t2 = pool.tile([P, HS, B], F32, tag="t2", bufs=2)
                        nc.vector.tensor_mul(t1[:], nR[:, 0:HS, :], cof(ew, l, 1, r))
                        nc.vector.tensor_add(t1[:], t1[:], cof(ew, l, 0, r))
                        nc.vector.tensor_mul(t2[:], nL[:, 0:HS, :], cof(ew, l, 2, r))
                        nc.vector.tensor_add(t1[:], t1[:], t2[:])
                        t2 = pool.tile([P, HS, B], F32, tag="t2", bufs=2)
                        nc.vector.tensor_mul(t2[:], u[:, 0:HS, :], cof(ew, l, 3, r))
                        Sr = pool.tile([P, HS, B], F32, tag=f"s{r}", bufs=2)
                        nc.vector.tensor_add(Sr[:], t1[:], t2[:])
                        S.append(Sr)
                    below = slice(1, HS + 1)
                    o = outt[:, 0:HS, :]
                    t3 = pool.tile([P, HS, B], F32, tag="t3", bufs=2)
                    nc.vector.tensor_mul(o, S[1][:], nR[:, below, :])
                    nc.vector.tensor_add(o, o, S[0][:])
                    nc.vector.tensor_mul(t3[:], S[2][:], nL[:, below, :])
                    nc.vector.tensor_add(o, o, t3[:])
                    t3 = pool.tile([P, HS, B], F32, tag="t3", bufs=2)
                    nc.vector.tensor_mul(t3[:], S[3][:], u[:, below, :])
                    nc.vector.tensor_add(o, o, t3[:])
                SE, SO = SEn, SOn

            # ---- layer 3: even rows of even cols only ----
            UE = pool.tile([P, HT, B], F32, tag="ue", bufs=2)
            nc.scalar.memzero(UE[:, HS:HT, :])
            nc.vector.tensor_mul(UE[:, 0:HS, :], SE[:, 0:HS, :], SO[:, 0:HS, :])
            ev = slice(0, 32, 2)   # rows 0,2,...,30
            od = slice(1, 33, 2)   # rows 1,3,...,31
            S = []
            for r in range(4):
                t1 = pool.tile([P, 16, B], F32, tag="f1", bufs=2)
                t2 = pool.tile([P, 16, B], F32, tag="f2", bufs=2)
                nc.vector.tensor_mul(t1[:], SO[:, ev, :], cof(0, 3, 1, r, ev))
                nc.vector.tensor_add(t1[:], t1[:], cof(0, 3, 0, r, ev))
                nc.vector.tensor_mul(t2[:], SE[:, ev, :], cof(0, 3, 2, r, ev))
                nc.vector.tensor_add(t1[:], t1[:], t2[:])
                t2 = pool.tile([P, 16, B], F32, tag="f2", bufs=2)
                nc.vector.tensor_mul(t2[:], UE[:, ev, :], cof(0, 3, 3, r, ev))
                Sr = pool.tile([P, 16, B], F32, tag=f"fs{r}")
                nc.vector.tensor_add(Sr[:], t1[:], t2[:])
                S.append(Sr)
            out_t = pool.tile([P, 16, B], F32, tag="out")
            t3 = pool.tile([P, 16, B], F32, tag="f3", bufs=2)
            nc.vector.tensor_mul(out_t[:], S[1][:], SO[:, od, :])
            nc.vector.tensor_add(out_t[:], out_t[:], S[0][:])
            nc.vector.tensor_mul(t3[:], S[2][:], SE[:, od, :])
            nc.vector.tensor_add(out_t[:], out_t[:], t3[:])
            t3 = pool.tile([P, 16, B], F32, tag="f3", bufs=2)
            nc.vector.tensor_mul(t3[:], S[3][:], UE[:, od, :])
            nc.vector.tensor_add(out_t[:], out_t[:], t3[:])
            nc.sync.dma_start(out=o_out[:], in_=out_t[:])

    nc.compile()
    return nc


def _get_nc():
    if "nc" not in _NC_CACHE:
        _NC_CACHE["nc"] = _build()
    return _NC_CACHE["nc"]


def _shard_inputs(x, toggle_gates):
    in_maps = []
    for i in range(NCORES):
        xrows = np.arange(16 * i, 16 * i + HX) % 128
        xs = np.ascontiguousarray(x[:, xrows, :].transpose(2, 1, 0))  # (w,h',b)
        grow = np.arange(32 * i, 32 * i + HS) % 256
        g = toggle_gates[:, :, grow, :].transpose(3, 0, 1, 2)  # (w,l,c,h)
        g = np.ascontiguousarray(g).reshape(P, 2, 4, 4, 4, HS)
        in_maps.append({"x": xs, "g": g})
    return in_maps


def kernel(x, toggle_gates):
    x = np.asarray(x, dtype=np.float32)
    toggle_gates = np.asarray(toggle_gates, dtype=np.float32)
    nc = _get_nc()
    in_maps = _shard_inputs(x, toggle_gates)
    res = run_bass_kernel_spmd(nc, in_maps, list(range(NCORES)))
    out = np.empty((B, 128, 128), np.float32)
    for i in range(NCORES):
        o = res.results[i]["o"]  # (128 w', 16 y, 32 b)
        out[:, 16 * i : 16 * i + 16, :] = o.transpose(2, 1, 0)
    return out


# revision 5
# speedup vs baseline: 1.8772x; 1.8772x over previous
"""Trainium2 Bass kernel for the 4-layer soft-logic-gate cellular automaton.

Hardcoded for x:(32,128,128) f32, toggle_gates:(4,16,256,256) f32, 8 cores.

Sharding: spatial over state rows H=256 with a redundant halo -> zero
cross-core communication. Core i consumes x rows [16i,16i+18) (mod 128),
gate rows [32i,32i+35) (mod 256), produces out rows [16i,16i+16).

Math: per pixel out = sum_c sigmoid(g_c) * prod_n(bit_n(c)? v_n : 1-v_n)
over the 2x2 torus neighborhood. Evaluated in the multilinear (Moebius)
basis: out = sum_{q,r} A_qr * n_q(row h) * n_r(row h+1),
n = [1, vR, vL, vL*vR]; A = 16 batch-free coefficient maps per layer
(on-device: ACT sigmoid -> 4 in-place DVE subtracts per layer).

Layout: states split by column parity into SE/SO tiles
[128 partitions = w', free = (h, b)], fp16 so tensor_tensor hits the DVE
2x_1P perf mode. w+1 for odd columns needs partition+1: materialized once
per layer via SBUF->SBUF DMA (engines cannot read across partitions).
Coefficients are x2-replicated innermost ([...,h,2]) so every operand has
a 16-bit step-1 4-byte-aligned innermost run (ISA allows max 3 free dims,
so the batch broadcast is (0,16),(1,2)). Pure adds are quad-fused over r.
Layer 0 collapses (3 of 4 neighbors zero); layer 3 computes only even
rows of even columns. The reference clip(0,1) is a mathematical no-op
(the truth-table weights are a partition of unity) and is dropped.
"""

import sys

sys.path.insert(0, "/opt/trn_rl_repo")

import numpy as np

import concourse.bacc as bacc
import concourse.mybir as mybir
from concourse.bass_utils import run_bass_kernel_spmd
from concourse.tile import TileContext

P = 128          # partitions = w' (column pairs)
B = 32           # batch
HS = 35          # local state rows (31 needed + 4 halo)
HG = 36          # gate-row storage (padded to even for 4B-aligned slices)
HX = 18          # local x rows
NCORES = 8
F32 = mybir.dt.float32
F16 = mybir.dt.float16

_NC_CACHE = {}


def _build():
    nc = bacc.Bacc("TRN2", target_bir_lowering=False, debug=False, num_devices=NCORES)
    x_in = nc.dram_tensor("x", [P, HX, B], F32, kind="ExternalInput")
    g_in = nc.dram_tensor("g", [P, 2, 4, 4, 4, HG], F32, kind="ExternalInput")
    o_out = nc.dram_tensor("o", [P, 16, B], F16, kind="ExternalOutput")

    with TileContext(nc) as tc:
        with tc.tile_pool(name="pool", bufs=1) as pool:
            # ---- gate pipeline: DMA -> sigmoid(fp16) -> Moebius -> x2-replicate
            gt = pool.tile([P, 2, 4, 4, 4, HG], F32, tag="gt")
            sg = pool.tile([P, 2, 4, 4, 4, HG], F16, tag="sg")
            rep = pool.tile([P, 2, 4, 4, 4, HG, 2], F16, tag="rep")

            def prep(l):
                nc.sync.dma_start(out=gt[:, :, l], in_=g_in[:, :, l])
                s = sg[:, :, l]
                nc.scalar.activation(
                    s, gt[:, :, l], mybir.ActivationFunctionType.Sigmoid
                )
                nc.vector.tensor_sub(  # a3: odd r -= even r
                    s[:, :, :, 1::2, :], s[:, :, :, 1::2, :], s[:, :, :, 0::2, :]
                )
                nc.vector.tensor_sub(  # a2: r {2,3} -= {0,1}
                    s[:, :, :, 2:4, :], s[:, :, :, 2:4, :], s[:, :, :, 0:2, :]
                )
                nc.vector.tensor_sub(  # a1: odd q -= even q
                    s[:, :, 1::2, :, :], s[:, :, 1::2, :, :], s[:, :, 0::2, :, :]
                )
                nc.vector.tensor_sub(  # a0: q {2,3} -= {0,1}
                    s[:, :, 2:4, :, :], s[:, :, 2:4, :, :], s[:, :, 0:2, :, :]
                )
                nc.scalar.copy(
                    rep[:, :, l],
                    s.unsqueeze(5).broadcast_to(list(s.shape) + [2]),
                )

            for l in range(4):
                prep(l)

            def cof(ew, l, q, r, rows=slice(0, HS)):
                # coeff view shaped [P, n, 16, 2] (b split so innermost is
                # a step-1 pair; the (0,16) broadcast sits in the middle)
                s = rep[:, ew, l, q, r, rows, :]          # [P, n, 2]
                n = s.shape[1]
                return s.unsqueeze(2).broadcast_to([P, n, B // 2, 2])

            def st2(ap):
                # [P, n, B] state/temp view -> [P, n, B//2, 2]
                return ap.rearrange("p h (c j) -> p h c j", j=2)

            # ---- x load, cast to fp16, shifted copy ----
            xt32 = pool.tile([P, HX, B], F32, tag="xt32")
            nc.sync.dma_start(out=xt32[:], in_=x_in[:])
            xt = pool.tile([P, HX, B], F16, tag="xt")
            nc.scalar.copy(xt[:], xt32[:])
            xs = pool.tile([P, HX, B], F16, tag="xs")
            nc.sync.dma_start(out=xs[0:127], in_=xt[1:128])
            nc.sync.dma_start(out=xs[127:128], in_=xt[0:1])

            # ---- layer 0: state_1 (rows 0..33) from zero-upsampled x ----
            SE = pool.tile([P, HS, B], F16, tag="se1")
            SO = pool.tile([P, HS, B], F16, tag="so1")
            ev_c, od_c = slice(0, 33, 2), slice(1, 34, 2)  # 17 rows each
            for (ew, outt, xsrc, xr, qa, ra, rows_o, rows_c) in (
                (0, SE, xt, slice(0, 17), 2, 0, slice(0, 34, 2), ev_c),
                (0, SE, xt, slice(1, 18), 0, 2, slice(1, 35, 2), od_c),
                (1, SO, xs, slice(0, 17), 1, 0, slice(0, 34, 2), ev_c),
                (1, SO, xs, slice(1, 18), 0, 1, slice(1, 35, 2), od_c),
            ):
                t0 = pool.tile([P, 17, B], F16, tag="l0t", bufs=2)
                nc.vector.tensor_mul(
                    st2(t0[:]), st2(xsrc[:, xr, :]), cof(ew, 0, qa, ra, rows_c)
                )
                nc.vector.tensor_add(
                    st2(outt[:, rows_o, :]), st2(t0[:]), cof(ew, 0, 0, 0, rows_c)
                )

            def inner(l, ew, nR, nL, u, rows, n):
                """Fused-over-r inner sums: S[r] = A0r + A1r*nR + A2r*nL + A3r*u.
                Coeff ops per r (3-free-dim APs); the two pure adds quad-fused."""
                T1 = pool.tile([P, 4, HS, B], F16, tag="T1", bufs=2)
                T2 = pool.tile([P, 4, HS, B], F16, tag="T2", bufs=2)
                T3 = pool.tile([P, 4, HS, B], F16, tag="T3", bufs=2)
                for r in range(4):
                    nc.vector.tensor_mul(
                        st2(T1[:, r, 0:n, :]), st2(nR), cof(ew, l, 1, r, rows)
                    )
                    nc.vector.tensor_add(
                        st2(T1[:, r, 0:n, :]), st2(T1[:, r, 0:n, :]), cof(ew, l, 0, r, rows)
                    )
                    nc.vector.tensor_mul(
                        st2(T2[:, r, 0:n, :]), st2(nL), cof(ew, l, 2, r, rows)
                    )
                    nc.vector.tensor_mul(
                        st2(T3[:, r, 0:n, :]), st2(u), cof(ew, l, 3, r, rows)
                    )
                nc.vector.tensor_add(
                    T1[:, :, 0:n, :], T1[:, :, 0:n, :], T2[:, :, 0:n, :]
                )
                nc.vector.tensor_add(
                    T1[:, :, 0:n, :], T1[:, :, 0:n, :], T3[:, :, 0:n, :]
                )
                return T1

            def outer(S, nRb, nLb, ub, o, n):
                """out = S0 + S1*nR' + S2*nL' + S3*u' (primes = row below)."""
                t3 = pool.tile([P, HS, B], F16, tag="t3", bufs=2)
                nc.vector.tensor_mul(o, S[:, 1, 0:n, :], nRb)
                nc.vector.tensor_add(o, o, S[:, 0, 0:n, :])
                nc.vector.tensor_mul(t3[:, 0:n, :], S[:, 2, 0:n, :], nLb)
                nc.vector.tensor_add(o, o, t3[:, 0:n, :])
                nc.vector.tensor_mul(t3[:, 0:n, :], S[:, 3, 0:n, :], ub)
                nc.vector.tensor_add(o, o, t3[:, 0:n, :])

            # ---- layers 1, 2 (full; layer l output has 34-l valid rows) ----
            for l in (1, 2):
                n = 34 - l          # output rows
                hv = n + 1          # valid input rows
                SESH = pool.tile([P, HS, B], F16, tag="sesh", bufs=2)
                nc.sync.dma_start(out=SESH[0:127, 0:hv, :], in_=SE[1:128, 0:hv, :])
                nc.sync.dma_start(out=SESH[127:128, 0:hv, :], in_=SE[0:1, 0:hv, :])
                UE = pool.tile([P, HS, B], F16, tag="ue", bufs=2)
                UO = pool.tile([P, HS, B], F16, tag="uo", bufs=2)
                nc.vector.tensor_mul(UE[:, 0:hv, :], SE[:, 0:hv, :], SO[:, 0:hv, :])
                nc.vector.tensor_mul(UO[:, 0:hv, :], SO[:, 0:hv, :], SESH[:, 0:hv, :])
                SEn = pool.tile([P, HS, B], F16, tag=f"se{l + 1}")
                SOn = pool.tile([P, HS, B], F16, tag=f"so{l + 1}")
                rows = slice(0, n)
                below = slice(1, n + 1)
                for ew, (nR, nL, u, outt) in enumerate(
                    [(SO, SE, UE, SEn), (SESH, SO, UO, SOn)]
                ):
                    S = inner(l, ew, nR[:, rows, :], nL[:, rows, :], u[:, rows, :], rows, n)
                    outer(
                        S,
                        nR[:, below, :],
                        nL[:, below, :],
                        u[:, below, :],
                        outt[:, rows, :],
                        n,
                    )
                SE, SO = SEn, SOn

            # ---- layer 3: even rows of even cols only ----
            hv = 32
            UE = pool.tile([P, HS, B], F16, tag="ue", bufs=2)
            nc.vector.tensor_mul(UE[:, 0:hv, :], SE[:, 0:hv, :], SO[:, 0:hv, :])
            ev = slice(0, 32, 2)   # 16 rows 0,2,...,30
            od = slice(1, 33, 2)   # 1,3,...,31
            evc = slice(0, 31, 2)  # coeff rows
            T1 = pool.tile([P, 4, 16, B], F16, tag="U1")
            T2 = pool.tile([P, 4, 16, B], F16, tag="U2")
            T3 = pool.tile([P, 4, 16, B], F16, tag="U3")
            for r in range(4):
                nc.vector.tensor_mul(st2(T1[:, r]), st2(SO[:, ev, :]), cof(0, 3, 1, r, evc))
                nc.vector.tensor_add(st2(T1[:, r]), st2(T1[:, r]), cof(0, 3, 0, r, evc))
                nc.vector.tensor_mul(st2(T2[:, r]), st2(SE[:, ev, :]), cof(0, 3, 2, r, evc))
                nc.vector.tensor_mul(st2(T3[:, r]), st2(UE[:, ev, :]), cof(0, 3, 3, r, evc))
            nc.vector.tensor_add(T1[:], T1[:], T2[:])
            nc.vector.tensor_add(T1[:], T1[:], T3[:])
            S = T1
            out_t = pool.tile([P, 16, B], F16, tag="out")
            t3 = pool.tile([P, 16, B], F16, tag="f3", bufs=2)
            nc.vector.tensor_mul(out_t[:], S[:, 1], SO[:, od, :])
            nc.vector.tensor_add(out_t[:], out_t[:], S[:, 0])
            nc.vector.tensor_mul(t3[:], S[:, 2], SE[:, od, :])
            nc.vector.tensor_add(out_t[:], out_t[:], t3[:])
            nc.vector.tensor_mul(t3[:], S[:, 3], UE[:, od, :])
            nc.vector.tensor_add(out_t[:], out_t[:], t3[:])
            nc.sync.dma_start(out=o_out[:], in_=out_t[:])

    nc.compile()
    return nc


def _get_nc():
    if "nc" not in _NC_CACHE:
        _NC_CACHE["nc"] = _build()
    return _NC_CACHE["nc"]


def _shard_inputs(x, toggle_gates):
    in_maps = []
    for i in range(NCORES):
        xrows = np.arange(16 * i, 16 * i + HX) % 128
        xs = np.ascontiguousarray(x[:, xrows, :].transpose(2, 1, 0))  # (w,h',b)
        grow = np.arange(32 * i, 32 * i + HS) % 256
        g = toggle_gates[:, :, grow, :].transpose(3, 0, 1, 2)  # (w,l,c,h)
        g = np.ascontiguousarray(g).reshape(P, 2, 4, 4, 4, HS)
        gp = np.zeros((P, 2, 4, 4, 4, HG), np.float32)
        gp[..., :HS] = g
        in_maps.append({"x": xs, "g": gp})
    return in_maps


def kernel(x, toggle_gates):
    x = np.asarray(x, dtype=np.float32)
    toggle_gates = np.asarray(toggle_gates, dtype=np.float32)
    nc = _get_nc()
    in_maps = _shard_inputs(x, toggle_gates)
    res = run_bass_kernel_spmd(nc, in_maps, list(range(NCORES)))
    out = np.empty((B, 128, 128), np.float32)
    for i in range(NCORES):
        o = res.results[i]["o"].astype(np.float32)  # (128 w', 16 y, 32 b)
        out[:, 16 * i : 16 * i + 16, :] = o.transpose(2, 1, 0)
    return out


# revision 11
# speedup vs baseline: 1.9824x; 1.0560x over previous
"""Trainium2 Bass kernel for the 4-layer soft-logic-gate cellular automaton.

Hardcoded for x:(32,128,128) f32, toggle_gates:(4,16,256,256) f32, 8 cores.

Sharding: spatial over state rows H=256 with a redundant halo -> zero
cross-core communication. Core i consumes x rows [16i,16i+18) (mod 128),
gate rows [32i,32i+35) (mod 256), produces out rows [16i,16i+16).

Math: per pixel out = sum_c sigmoid(g_c) * prod_n(bit_n(c)? v_n : 1-v_n)
over the 2x2 torus neighborhood. Evaluated in the multilinear (Moebius)
basis: out = sum_{q,r} A_qr * n_q(row h) * n_r(row h+1),
n = [1, vR, vL, vL*vR]; A = 16 batch-free coefficient maps per layer
(on-device: ACT sigmoid -> 4 in-place DVE subtracts per layer).

Layout: states split by column parity into SE/SO tiles
[128 partitions = w', free = (h, b)], fp16 so tensor_tensor hits the DVE
2x_1P perf mode. w+1 for odd columns needs partition+1: materialized once
per layer via SBUF->SBUF DMA (engines cannot read across partitions).
Coefficients are x2-replicated innermost ([...,h,2]) so every operand has
a 16-bit step-1 4-byte-aligned innermost run (ISA allows max 3 free dims,
so the batch broadcast is (0,16),(1,2)). Pure adds are quad-fused over r.
Layer 0 collapses (3 of 4 neighbors zero); layer 3 computes only even
rows of even columns. The reference clip(0,1) is a mathematical no-op
(the truth-table weights are a partition of unity) and is dropped.
"""

import sys

sys.path.insert(0, "/opt/trn_rl_repo")

import numpy as np

import concourse.bacc as bacc
import concourse.mybir as mybir
from concourse.bass_utils import run_bass_kernel_spmd
from concourse.tile import TileContext

P = 128          # partitions = w' (column pairs)
B = 32           # batch
HS = 35          # local state rows (31 needed + 4 halo)
HG = 36          # gate-row storage (padded to even for 4B-aligned slices)
HX = 18          # local x rows
NCORES = 8
F32 = mybir.dt.float32
F16 = mybir.dt.float16

_NC_CACHE = {}


def _build():
    nc = bacc.Bacc("TRN2", target_bir_lowering=False, debug=False, num_devices=NCORES)
    x_in = nc.dram_tensor("x", [P, HX, B], F32, kind="ExternalInput")
    g_in = nc.dram_tensor("g", [P, 2, 4, 4, 4, HG], F32, kind="ExternalInput")
    o_out = nc.dram_tensor("o", [P, 16, B], F16, kind="ExternalOutput")

    with TileContext(nc) as tc:
        with tc.tile_pool(name="pool", bufs=1) as pool:
            # ---- gate pipeline: DMA -> sigmoid(fp16) -> Moebius -> x2-replicate
            gt = pool.tile([P, 2, 4, 4, 4, HG], F32, tag="gt")
            sg = pool.tile([P, 2, 4, 4, 4, HG], F16, tag="sg")
            rep = pool.tile([P, 2, 4, 4, 4, HG, 2], F16, tag="rep")

            def prep(l, ews=(slice(0, 2),)):
                for ewsl in ews:
                    nc.sync.dma_start(out=gt[:, ewsl, l], in_=g_in[:, ewsl, l])
                    s = sg[:, ewsl, l]
                    nc.scalar.activation(
                        s, gt[:, ewsl, l], mybir.ActivationFunctionType.Sigmoid
                    )
                    nc.vector.tensor_sub(  # a3: odd r -= even r
                        s[:, :, :, 1::2, :], s[:, :, :, 1::2, :], s[:, :, :, 0::2, :]
                    )
                    nc.vector.tensor_sub(  # a2: r {2,3} -= {0,1}
                        s[:, :, :, 2:4, :], s[:, :, :, 2:4, :], s[:, :, :, 0:2, :]
                    )
                    nc.vector.tensor_sub(  # a1: odd q -= even q
                        s[:, :, 1::2, :, :], s[:, :, 1::2, :, :], s[:, :, 0::2, :, :]
                    )
                    nc.vector.tensor_sub(  # a0: q {2,3} -= {0,1}
                        s[:, :, 2:4, :, :], s[:, :, 2:4, :, :], s[:, :, 0:2, :, :]
                    )
                    nc.scalar.copy(
                        rep[:, ewsl, l],
                        s.unsqueeze(5).broadcast_to(list(s.shape) + [2]),
                    )

            def cof(ew, l, q, r, rows=slice(0, HS)):
                # coeff view shaped [P, n, 16, 2] (b split so innermost is
                # a step-1 pair; the (0,16) broadcast sits in the middle)
                s = rep[:, ew, l, q, r, rows, :]          # [P, n, 2]
                n = s.shape[1]
                return s.unsqueeze(2).broadcast_to([P, n, B // 2, 2])

            def st2(ap):
                # [P, n, B] state/temp view -> [P, n, B//2, 2]
                return ap.rearrange("p h (c j) -> p h c j", j=2)

            # ---- x load, cast to fp16, shifted copy ----
            xt32 = pool.tile([P, HX, B], F32, tag="xt32")
            nc.sync.dma_start(out=xt32[:], in_=x_in[:])
            xt = pool.tile([P, HX, B], F16, tag="xt")
            nc.scalar.copy(xt[:], xt32[:])
            xs = pool.tile([P, HX, B], F16, tag="xs")
            nc.sync.dma_start(out=xs[0:127], in_=xt[1:128])
            nc.sync.dma_start(out=xs[127:128], in_=xt[0:1])

            prep(0, ews=(slice(0, 1), slice(1, 2)))

            # ---- layer 0: state_1 (rows 0..33) from zero-upsampled x ----
            SE = pool.tile([P, HS, B], F16, tag="se1")
            SO = pool.tile([P, HS, B], F16, tag="so1")
            ev_c, od_c = slice(0, 33, 2), slice(1, 34, 2)  # 17 rows each
            for (ew, outt, xsrc, xr, qa, ra, rows_o, rows_c) in (
                (0, SE, xt, slice(0, 17), 2, 0, slice(0, 34, 2), ev_c),
                (0, SE, xt, slice(1, 18), 0, 2, slice(1, 35, 2), od_c),
                (1, SO, xs, slice(0, 17), 1, 0, slice(0, 34, 2), ev_c),
                (1, SO, xs, slice(1, 18), 0, 1, slice(1, 35, 2), od_c),
            ):
                t0 = pool.tile([P, 17, B], F16, tag="l0t", bufs=2)
                nc.vector.tensor_mul(
                    st2(t0[:]), st2(xsrc[:, xr, :]), cof(ew, 0, qa, ra, rows_c)
                )
                nc.vector.tensor_add(
                    st2(outt[:, rows_o, :]), st2(t0[:]), cof(ew, 0, 0, 0, rows_c)
                )

            def inner(l, ew, nR, nL, u, rows, n):
                """Fused-over-r inner sums: S[r] = A0r + A1r*nR + A2r*nL + A3r*u.
                Independent muls first, dependent adds later (fills DVE pipe)."""
                T1 = pool.tile([P, 4, HS, B], F16, tag="T1", bufs=2)
                T2 = pool.tile([P, 4, HS, B], F16, tag="T2", bufs=2)
                T3 = pool.tile([P, 4, HS, B], F16, tag="T3", bufs=2)
                for r in range(4):
                    nc.vector.tensor_mul(
                        st2(T1[:, r, 0:n, :]), st2(nR), cof(ew, l, 1, r, rows)
                    )
                for r in range(4):
                    nc.vector.tensor_mul(
                        st2(T2[:, r, 0:n, :]), st2(nL), cof(ew, l, 2, r, rows)
                    )
                for r in range(4):
                    nc.vector.tensor_mul(
                        st2(T3[:, r, 0:n, :]), st2(u), cof(ew, l, 3, r, rows)
                    )
                for r in range(4):
                    nc.vector.tensor_add(
                        st2(T1[:, r, 0:n, :]), st2(T1[:, r, 0:n, :]), cof(ew, l, 0, r, rows)
                    )
                nc.vector.tensor_add(
                    T2[:, :, 0:n, :], T2[:, :, 0:n, :], T3[:, :, 0:n, :]
                )
                nc.vector.tensor_add(
                    T1[:, :, 0:n, :], T1[:, :, 0:n, :], T2[:, :, 0:n, :]
                )
                return T1

            def outer(S, nRb, nLb, ub, o, n, ew=0):
                """out = (S0 + S1*nR') + (S2*nL' + S3*u') as a balanced tree."""
                ta = pool.tile([P, HS, B], F16, tag="ta", bufs=2)
                tb = pool.tile([P, HS, B], F16, tag="tb", bufs=2)
                nc.vector.tensor_mul(ta[:, 0:n, :], S[:, 1, 0:n, :], nRb)
                nc.vector.tensor_mul(tb[:, 0:n, :], S[:, 2, 0:n, :], nLb)
                nc.vector.tensor_mul(o, S[:, 3, 0:n, :], ub)
                nc.vector.tensor_add(ta[:, 0:n, :], ta[:, 0:n, :], S[:, 0, 0:n, :])
                nc.vector.tensor_add(o, o, tb[:, 0:n, :])
                nc.vector.tensor_add(o, o, ta[:, 0:n, :])

            # ---- layers 1, 2 (full; layer l output has 34-l valid rows) ----
            for l in (1, 2):
                prep(l)
                n = 34 - l          # output rows
                hv = n + 1          # valid input rows
                SESH = pool.tile([P, HS, B], F16, tag="sesh", bufs=2)
                nc.sync.dma_start(out=SESH[0:127, 0:hv, :], in_=SE[1:128, 0:hv, :])
                nc.sync.dma_start(out=SESH[127:128, 0:hv, :], in_=SE[0:1, 0:hv, :])
                UE = pool.tile([P, HS, B], F16, tag="ue", bufs=2)
                UO = pool.tile([P, HS, B], F16, tag="uo", bufs=2)
                nc.vector.tensor_mul(UE[:, 0:hv, :], SE[:, 0:hv, :], SO[:, 0:hv, :])
                nc.vector.tensor_mul(UO[:, 0:hv, :], SO[:, 0:hv, :], SESH[:, 0:hv, :])
                SEn = pool.tile([P, HS, B], F16, tag=f"se{l + 1}")
                SOn = pool.tile([P, HS, B], F16, tag=f"so{l + 1}")
                rows = slice(0, n)
                below = slice(1, n + 1)
                for ew, (nR, nL, u, outt) in enumerate(
                    [(SO, SE, UE, SEn), (SESH, SO, UO, SOn)]
                ):
                    S = inner(l, ew, nR[:, rows, :], nL[:, rows, :], u[:, rows, :], rows, n)
                    outer(
                        S,
                        nR[:, below, :],
                        nL[:, below, :],
                        u[:, below, :],
                        outt[:, rows, :],
                        n,
                        ew,
                    )
                SE, SO = SEn, SOn

            # ---- layer 3: even rows of even cols only ----
            prep(3)
            hv = 32
            UE = pool.tile([P, HS, B], F16, tag="ue", bufs=2)
            nc.vector.tensor_mul(UE[:, 0:hv, :], SE[:, 0:hv, :], SO[:, 0:hv, :])
            ev = slice(0, 32, 2)   # 16 rows 0,2,...,30
            od = slice(1, 33, 2)   # 1,3,...,31
            evc = slice(0, 31, 2)  # coeff rows
            T1 = pool.tile([P, 4, 16, B], F16, tag="U1")
            T2 = pool.tile([P, 4, 16, B], F16, tag="U2")
            T3 = pool.tile([P, 4, 16, B], F16, tag="U3")
            for r in range(4):
                nc.vector.tensor_mul(st2(T1[:, r]), st2(SO[:, ev, :]), cof(0, 3, 1, r, evc))
                nc.vector.tensor_add(st2(T1[:, r]), st2(T1[:, r]), cof(0, 3, 0, r, evc))
                nc.vector.tensor_mul(st2(T2[:, r]), st2(SE[:, ev, :]), cof(0, 3, 2, r, evc))
                nc.vector.tensor_mul(st2(T3[:, r]), st2(UE[:, ev, :]), cof(0, 3, 3, r, evc))
            nc.vector.tensor_add(T1[:], T1[:], T2[:])
            nc.vector.tensor_add(T1[:], T1[:], T3[:])
            S = T1
            out_t = pool.tile([P, 16, B], F16, tag="out")
            t3 = pool.tile([P, 16, B], F16, tag="f3", bufs=2)
            nc.vector.tensor_mul(out_t[:], S[:, 1], SO[:, od, :])
            nc.vector.tensor_add(out_t[:], out_t[:], S[:, 0])
            nc.vector.tensor_mul(t3[:], S[:, 2], SE[:, od, :])
            nc.vector.tensor_add(out_t[:], out_t[:], t3[:])
            nc.vector.tensor_mul(t3[:], S[:, 3], UE[:, od, :])
            nc.vector.tensor_add(out_t[:], out_t[:], t3[:])
            nc.sync.dma_start(out=o_out[:], in_=out_t[:])

    nc.compile()
    return nc


def _get_nc():
    if "nc" not in _NC_CACHE:
        _NC_CACHE["nc"] = _build()
    return _NC_CACHE["nc"]


def _shard_inputs(x, toggle_gates):
    in_maps = []
    for i in range(NCORES):
        xrows = np.arange(16 * i, 16 * i + HX) % 128
        xs = np.ascontiguousarray(x[:, xrows, :].transpose(2, 1, 0))  # (w,h',b)
        grow = np.arange(32 * i, 32 * i + HS) % 256
        g = toggle_gates[:, :, grow, :].transpose(3, 0, 1, 2)  # (w,l,c,h)
        g = np.ascontiguousarray(g).reshape(P, 2, 4, 4, 4, HS)
        gp = np.zeros((P, 2, 4, 4, 4, HG), np.float32)
        gp[..., :HS] = g
        in_maps.append({"x": xs, "g": gp})
    return in_maps


def kernel(x, toggle_gates):
    x = np.asarray(x, dtype=np.float32)
    toggle_gates = np.asarray(toggle_gates, dtype=np.float32)
    nc = _get_nc()
    in_maps = _shard_inputs(x, toggle_gates)
    res = run_bass_kernel_spmd(nc, in_maps, list(range(NCORES)))
    out = np.empty((B, 128, 128), np.float32)
    for i in range(NCORES):
        o = res.results[i]["o"].astype(np.float32)  # (128 w', 16 y, 32 b)
        out[:, 16 * i : 16 * i + 16, :] = o.transpose(2, 1, 0)
    return out


# revision 12
# speedup vs baseline: 1.9920x; 1.0048x over previous
"""Trainium2 Bass kernel for the 4-layer soft-logic-gate cellular automaton.

Hardcoded for x:(32,128,128) f32, toggle_gates:(4,16,256,256) f32, 8 cores.

Sharding: spatial over state rows H=256 with a redundant halo -> zero
cross-core communication. Core i consumes x rows [16i,16i+18) (mod 128),
gate rows [32i,32i+35) (mod 256), produces out rows [16i,16i+16).

Math: per pixel out = sum_c sigmoid(g_c) * prod_n(bit_n(c)? v_n : 1-v_n)
over the 2x2 torus neighborhood. Evaluated in the multilinear (Moebius)
basis: out = sum_{q,r} A_qr * n_q(row h) * n_r(row h+1),
n = [1, vR, vL, vL*vR]; A = 16 batch-free coefficient maps per layer
(on-device: ACT sigmoid -> 4 in-place DVE subtracts per layer).

Layout: states split by column parity into SE/SO tiles
[128 partitions = w', free = (h, b)], fp16 so tensor_tensor hits the DVE
2x_1P perf mode. w+1 for odd columns needs partition+1: materialized once
per layer via SBUF->SBUF DMA (engines cannot read across partitions).
Coefficients are x2-replicated innermost ([...,h,2]) so every operand has
a 16-bit step-1 4-byte-aligned innermost run (ISA allows max 3 free dims,
so the batch broadcast is (0,16),(1,2)). Pure adds are quad-fused over r.
Layer 0 collapses (3 of 4 neighbors zero); layer 3 computes only even
rows of even columns. The reference clip(0,1) is a mathematical no-op
(the truth-table weights are a partition of unity) and is dropped.
"""

import sys

sys.path.insert(0, "/opt/trn_rl_repo")

import numpy as np

import concourse.bacc as bacc
import concourse.mybir as mybir
from concourse.bass_utils import run_bass_kernel_spmd
from concourse.tile import TileContext

P = 128          # partitions = w' (column pairs)
B = 32           # batch
HS = 35          # local state rows (31 needed + 4 halo)
HG = 36          # gate-row storage (padded to even for 4B-aligned slices)
HX = 18          # local x rows
NCORES = 8
F32 = mybir.dt.float32
F16 = mybir.dt.float16

_NC_CACHE = {}


def _build():
    nc = bacc.Bacc("TRN2", target_bir_lowering=False, debug=False, num_devices=NCORES)
    x_in = nc.dram_tensor("x", [P, HX, B], F32, kind="ExternalInput")
    g_in = nc.dram_tensor("g", [P, 2, 4, 4, 4, HG], F32, kind="ExternalInput")
    o_out = nc.dram_tensor("o", [P, 16, B], F16, kind="ExternalOutput")

    with TileContext(nc) as tc:
        with tc.tile_pool(name="pool", bufs=1) as pool:
            # ---- gate pipeline: DMA -> sigmoid(fp16) -> Moebius -> x2-replicate
            gt = pool.tile([P, 2, 4, 4, 4, HG], F32, tag="gt")
            sg = pool.tile([P, 2, 4, 4, 4, HG], F16, tag="sg")
            rep = pool.tile([P, 2, 4, 4, 4, HG, 2], F16, tag="rep")

            def prep(l, ews=(slice(0, 2),)):
                for ewsl in ews:
                    nc.sync.dma_start(out=gt[:, ewsl, l], in_=g_in[:, ewsl, l])
                    s = sg[:, ewsl, l]
                    nc.scalar.activation(
                        s, gt[:, ewsl, l], mybir.ActivationFunctionType.Sigmoid
                    )
                    nc.vector.tensor_sub(  # a3: odd r -= even r
                        s[:, :, :, 1::2, :], s[:, :, :, 1::2, :], s[:, :, :, 0::2, :]
                    )
                    nc.vector.tensor_sub(  # a2: r {2,3} -= {0,1}
                        s[:, :, :, 2:4, :], s[:, :, :, 2:4, :], s[:, :, :, 0:2, :]
                    )
                    nc.vector.tensor_sub(  # a1: odd q -= even q
                        s[:, :, 1::2, :, :], s[:, :, 1::2, :, :], s[:, :, 0::2, :, :]
                    )
                    nc.vector.tensor_sub(  # a0: q {2,3} -= {0,1}
                        s[:, :, 2:4, :, :], s[:, :, 2:4, :, :], s[:, :, 0:2, :, :]
                    )
                    nc.scalar.copy(
                        rep[:, ewsl, l],
                        s.unsqueeze(5).broadcast_to(list(s.shape) + [2]),
                    )

            def cof(ew, l, q, r, rows=slice(0, HS)):
                # coeff view shaped [P, n, 16, 2] (b split so innermost is
                # a step-1 pair; the (0,16) broadcast sits in the middle)
                s = rep[:, ew, l, q, r, rows, :]          # [P, n, 2]
                n = s.shape[1]
                return s.unsqueeze(2).broadcast_to([P, n, B // 2, 2])

            def st2(ap):
                # [P, n, B] state/temp view -> [P, n, B//2, 2]
                return ap.rearrange("p h (c j) -> p h c j", j=2)

            # ---- x load (cast via gpsimd DMA), shifted copy ----
            xt = pool.tile([P, HX, B], F16, tag="xt")
            nc.gpsimd.dma_start(out=xt[:], in_=x_in[:])
            xs = pool.tile([P, HX, B], F16, tag="xs")
            nc.sync.dma_start(out=xs[0:127], in_=xt[1:128])
            nc.sync.dma_start(out=xs[127:128], in_=xt[0:1])

            prep(0, ews=(slice(0, 1), slice(1, 2)))

            # ---- layer 0: state_1 (rows 0..33) from zero-upsampled x ----
            SE = pool.tile([P, HS, B], F16, tag="se1")
            SO = pool.tile([P, HS, B], F16, tag="so1")
            ev_c, od_c = slice(0, 33, 2), slice(1, 34, 2)  # 17 rows each
            for (ew, outt, xsrc, xr, qa, ra, rows_o, rows_c) in (
                (0, SE, xt, slice(0, 17), 2, 0, slice(0, 34, 2), ev_c),
                (0, SE, xt, slice(1, 18), 0, 2, slice(1, 35, 2), od_c),
                (1, SO, xs, slice(0, 17), 1, 0, slice(0, 34, 2), ev_c),
                (1, SO, xs, slice(1, 18), 0, 1, slice(1, 35, 2), od_c),
            ):
                t0 = pool.tile([P, 17, B], F16, tag="l0t", bufs=2)
                nc.vector.tensor_mul(
                    st2(t0[:]), st2(xsrc[:, xr, :]), cof(ew, 0, qa, ra, rows_c)
                )
                nc.vector.tensor_add(
                    st2(outt[:, rows_o, :]), st2(t0[:]), cof(ew, 0, 0, 0, rows_c)
                )

            def inner(l, ew, nR, nL, u, rows, n):
                """Fused-over-r inner sums: S[r] = A0r + A1r*nR + A2r*nL + A3r*u.
                Independent muls first, dependent adds later (fills DVE pipe)."""
                T1 = pool.tile([P, 4, HS, B], F16, tag="T1", bufs=2)
                T2 = pool.tile([P, 4, HS, B], F16, tag="T2", bufs=2)
                T3 = pool.tile([P, 4, HS, B], F16, tag="T3", bufs=2)
                for r in range(4):
                    nc.vector.tensor_mul(
                        st2(T1[:, r, 0:n, :]), st2(nR), cof(ew, l, 1, r, rows)
                    )
                for r in range(4):
                    nc.vector.tensor_mul(
                        st2(T2[:, r, 0:n, :]), st2(nL), cof(ew, l, 2, r, rows)
                    )
                for r in range(4):
                    nc.vector.tensor_mul(
                        st2(T3[:, r, 0:n, :]), st2(u), cof(ew, l, 3, r, rows)
                    )
                for r in range(4):
                    nc.vector.tensor_add(
                        st2(T1[:, r, 0:n, :]), st2(T1[:, r, 0:n, :]), cof(ew, l, 0, r, rows)
                    )
                nc.vector.tensor_add(
                    T2[:, :, 0:n, :], T2[:, :, 0:n, :], T3[:, :, 0:n, :]
                )
                nc.vector.tensor_add(
                    T1[:, :, 0:n, :], T1[:, :, 0:n, :], T2[:, :, 0:n, :]
                )
                return T1

            def outer(S, nRb, nLb, ub, o, n, ew=0):
                """out = (S0 + S1*nR') + (S2*nL' + S3*u') as a balanced tree."""
                ta = pool.tile([P, HS, B], F16, tag="ta", bufs=2)
                tb = pool.tile([P, HS, B], F16, tag="tb", bufs=2)
                nc.vector.tensor_mul(ta[:, 0:n, :], S[:, 1, 0:n, :], nRb)
                nc.vector.tensor_mul(tb[:, 0:n, :], S[:, 2, 0:n, :], nLb)
                nc.vector.tensor_mul(o, S[:, 3, 0:n, :], ub)
                nc.vector.tensor_add(ta[:, 0:n, :], ta[:, 0:n, :], S[:, 0, 0:n, :])
                nc.vector.tensor_add(o, o, tb[:, 0:n, :])
                nc.vector.tensor_add(o, o, ta[:, 0:n, :])

            # ---- layers 1, 2 (full; layer l output has 34-l valid rows) ----
            for l in (1, 2):
                prep(l)
                n = 34 - l          # output rows
                hv = n + 1          # valid input rows
                SESH = pool.tile([P, HS, B], F16, tag="sesh", bufs=2)
                nc.sync.dma_start(out=SESH[0:127, 0:hv, :], in_=SE[1:128, 0:hv, :])
                nc.sync.dma_start(out=SESH[127:128, 0:hv, :], in_=SE[0:1, 0:hv, :])
                UE = pool.tile([P, HS, B], F16, tag="ue", bufs=2)
                UO = pool.tile([P, HS, B], F16, tag="uo", bufs=2)
                nc.vector.tensor_mul(UE[:, 0:hv, :], SE[:, 0:hv, :], SO[:, 0:hv, :])
                nc.vector.tensor_mul(UO[:, 0:hv, :], SO[:, 0:hv, :], SESH[:, 0:hv, :])
                SEn = pool.tile([P, HS, B], F16, tag=f"se{l + 1}")
                SOn = pool.tile([P, HS, B], F16, tag=f"so{l + 1}")
                rows = slice(0, n)
                below = slice(1, n + 1)
                for ew, (nR, nL, u, outt) in enumerate(
                    [(SO, SE, UE, SEn), (SESH, SO, UO, SOn)]
                ):
                    S = inner(l, ew, nR[:, rows, :], nL[:, rows, :], u[:, rows, :], rows, n)
                    outer(
                        S,
                        nR[:, below, :],
                        nL[:, below, :],
                        u[:, below, :],
                        outt[:, rows, :],
                        n,
                        ew,
                    )
                SE, SO = SEn, SOn

            # ---- layer 3: even rows of even cols only ----
            prep(3)
            hv = 32
            UE = pool.tile([P, HS, B], F16, tag="ue", bufs=2)
            nc.vector.tensor_mul(UE[:, 0:hv, :], SE[:, 0:hv, :], SO[:, 0:hv, :])
            ev = slice(0, 32, 2)   # 16 rows 0,2,...,30
            od = slice(1, 33, 2)   # 1,3,...,31
            evc = slice(0, 31, 2)  # coeff rows
            T1 = pool.tile([P, 4, 16, B], F16, tag="U1")
            T2 = pool.tile([P, 4, 16, B], F16, tag="U2")
            T3 = pool.tile([P, 4, 16, B], F16, tag="U3")
            for r in range(4):
                nc.vector.tensor_mul(st2(T1[:, r]), st2(SO[:, ev, :]), cof(0, 3, 1, r, evc))
                nc.vector.tensor_add(st2(T1[:, r]), st2(T1[:, r]), cof(0, 3, 0, r, evc))
                nc.vector.tensor_mul(st2(T2[:, r]), st2(SE[:, ev, :]), cof(0, 3, 2, r, evc))
                nc.vector.tensor_mul(st2(T3[:, r]), st2(UE[:, ev, :]), cof(0, 3, 3, r, evc))
            nc.vector.tensor_add(T1[:], T1[:], T2[:])
            nc.vector.tensor_add(T1[:], T1[:], T3[:])
            S = T1
            out_t = pool.tile([P, 16, B], F16, tag="out")
            t3 = pool.tile([P, 16, B], F16, tag="f3", bufs=2)
            nc.vector.tensor_mul(out_t[:], S[:, 1], SO[:, od, :])
            nc.vector.tensor_add(out_t[:], out_t[:], S[:, 0])
            nc.vector.tensor_mul(t3[:], S[:, 2], SE[:, od, :])
            nc.vector.tensor_add(out_t[:], out_t[:], t3[:])
            nc.vector.tensor_mul(t3[:], S[:, 3], UE[:, od, :])
            nc.vector.tensor_add(out_t[:], out_t[:], t3[:])
            nc.sync.dma_start(out=o_out[:], in_=out_t[:])

    nc.compile()
    return nc


def _get_nc():
    if "nc" not in _NC_CACHE:
        _NC_CACHE["nc"] = _build()
    return _NC_CACHE["nc"]


def _shard_inputs(x, toggle_gates):
    in_maps = []
    for i in range(NCORES):
        xrows = np.arange(16 * i, 16 * i + HX) % 128
        xs = np.ascontiguousarray(x[:, xrows, :].transpose(2, 1, 0))  # (w,h',b)
        grow = np.arange(32 * i, 32 * i + HS) % 256
        g = toggle_gates[:, :, grow, :].transpose(3, 0, 1, 2)  # (w,l,c,h)
        g = np.ascontiguousarray(g).reshape(P, 2, 4, 4, 4, HS)
        gp = np.zeros((P, 2, 4, 4, 4, HG), np.float32)
        gp[..., :HS] = g
        in_maps.append({"x": xs, "g": gp})
    return in_maps


def kernel(x, toggle_gates):
    x = np.asarray(x, dtype=np.float32)
    toggle_gates = np.asarray(toggle_gates, dtype=np.float32)
    nc = _get_nc()
    in_maps = _shard_inputs(x, toggle_gates)
    res = run_bass_kernel_spmd(nc, in_maps, list(range(NCORES)))
    out = np.empty((B, 128, 128), np.float32)
    for i in range(NCORES):
        o = res.results[i]["o"].astype(np.float32)  # (128 w', 16 y, 32 b)
        out[:, 16 * i : 16 * i + 16, :] = o.transpose(2, 1, 0)
    return out
